# revision 2
# baseline (speedup 1.0000x reference)
"""Trainium2 Bass kernel for the Tacotron-style LSTM decoder (nn_Decoder).

Self-contained: takes FULL unsharded inputs, returns the FULL output.

Strategy (8 NeuronCores, one chip):
  - The LSTM hidden/gate dimension is sharded 8 ways: core k owns hidden
    units [128k, 128k+128) and the matching 512 gate rows of w_ih/w_hh
    (reordered i,f,o,g so the sigmoid block is contiguous).
  - Phase 1 (parallel over T): prenet + input-projection gates_in
    computed with weights-stationary float32r matmuls, bias folded in,
    streamed to DRAM.
  - Phase 2 (sequential, T=800 steps): per step each core computes its
    512 gates as gates = g_in + W_hh_slice @ h_{t-1} (weights as the
    moving operand at full PE rate), applies sigmoid/tanh on ACT,
    the cell update on DVE (in transposed [hidden, batch] layout via PE
    transposes), and broadcasts its h-slice [128,32] to all 8 cores with
    remote_dma_broadcast (SBUF->SBUF, slot = sender id). Parity-split
    semaphores make the per-step all-gather race-free.
  - Phase 3: each core projects its T/8 chunk of mel from the h history
    + memory (proj weights replicated), so output assembly is pure
    concatenation.
"""

import numpy as np

N_MEL, ATTN, PRENET, RNN = 80, 512, 256, 1024
B, NCORE, GS = 32, 8, 512
T_FULL = 800

_nc_cache = {}


def _build(T):
    from contextlib import ExitStack

    import concourse.bass as bass
    import concourse.bacc as bacc
    import concourse.mybir as mybir

    F32 = mybir.dt.float32
    F32R = mybir.dt.float32r
    AF = mybir.ActivationFunctionType
    DS = bass.ds

    def r32(x):
        return x.bitcast(F32R)

    assert T % 16 == 0 and T % NCORE == 0
    R = T * B
    NBLK = R // 512
    TC = T // NCORE
    P3N = (TC + 15) // 16

    nc = bacc.Bacc("TRN2")

    prevT = nc.dram_tensor("prevT", [80, R], F32R, kind="ExternalInput")
    memT = nc.dram_tensor("memT", [ATTN, R], F32R, kind="ExternalInput")
    pw1T = nc.dram_tensor("pw1T", [80, PRENET], F32R, kind="ExternalInput")
    pw2T = nc.dram_tensor("pw2T", [PRENET, PRENET], F32R, kind="ExternalInput")
    wihT = nc.dram_tensor("wihT", [768, GS], F32R, kind="ExternalInput")
    whhT = nc.dram_tensor("whhT", [RNN, GS], F32R, kind="ExternalInput")
    biasg = nc.dram_tensor("biasg", [128, 4], F32, kind="ExternalInput")
    projT = nc.dram_tensor("projT", [RNN + ATTN, N_MEL], F32R, kind="ExternalInput")
    projb = nc.dram_tensor("projb", [N_MEL, 1], F32, kind="ExternalInput")
    ident = nc.dram_tensor("ident", [128, 128], F32R, kind="ExternalInput")
    cid = nc.dram_tensor("core_id", [1, 1], mybir.dt.uint32, kind="ExternalInput")
    melT = nc.dram_tensor("melT", [N_MEL, TC * B], F32, kind="ExternalOutput")

    g_dram = nc.dram_tensor("g_dram", [4, 128, R], F32R)
    hs_dram = nc.dram_tensor("hs_dram", [128, T, 256], F32R)

    with ExitStack() as ctx:
        def sb(name, shape, dtype=F32):
            return ctx.enter_context(nc.sbuf_tensor(name, shape, dtype))

        def ps(name, shape, dtype=F32):
            return ctx.enter_context(nc.psum_tensor(name, shape, dtype))

        def sem(name):
            return ctx.enter_context(nc.semaphore(name))

        sb_whh = [sb(f"whh{k}", [128, GS], F32R) for k in range(8)]
        sb_wih = [sb(f"wih{k}", [128, GS], F32R) for k in range(6)]
        sb_pw1 = sb("pw1", [80, PRENET], F32R)
        sb_pw2 = [sb(f"pw2_{k}", [128, PRENET], F32R) for k in range(2)]
        sb_proj = sb("proj", [128, 12 * N_MEL], F32R)
        sb_biasg = sb("sb_biasg", [128, 4])
        sb_projb = sb("sb_projb", [N_MEL, 1])
        sb_ident = sb("sb_ident", [128, 128], F32R)
        sb_cid = sb("sb_cid", [1, 1], mybir.dt.uint32)
        sb_prev = [sb(f"p1prev{j}", [80, 512], F32R) for j in range(2)]
        sb_mem = [[sb(f"p1mem{j}_{k}", [128, 512], F32R) for k in range(4)] for j in range(2)]
        sb_h1 = [[sb(f"p1h1_{j}_{m}", [128, 512], F32R) for m in range(2)] for j in range(2)]
        sb_h2 = [[sb(f"p1h2_{j}_{m}", [128, 512], F32R) for m in range(2)] for j in range(2)]
        sb_gout = [[sb(f"p1g{j}_{m}", [128, 512], F32R) for m in range(4)] for j in range(2)]
        sb_g = [[sb(f"g{j}_{m}", [128, 512], F32R) for m in range(4)] for j in range(2)]
        sb_hall = [sb(f"hall{j}", [128, 256], F32R) for j in range(2)]
        sb_sig = sb("sig", [B, 384], F32R)
        sb_tg = sb("tg", [B, 128])
        sb_t2u = sb("t2u", [B, 128], F32R)
        sb_m1 = sb("m1", [128, B])
        sb_c = [sb(f"c{j}", [128, B]) for j in range(2)]
        sb_d = sb("d", [128, B])
        sb_hT = [sb(f"hT{j}", [128, B], F32R) for j in range(2)]
        sb_hs = [sb(f"p3hs{j}", [128, 16 * 256], F32R) for j in range(2)]
        sb_m3 = [[sb(f"p3mem{j}_{k}", [128, 512], F32R) for k in range(4)] for j in range(2)]
        sb_mel = [sb(f"p3mel{j}", [N_MEL, 512]) for j in range(2)]

        ps_l1 = [ps(f"psl1_{m}", [128, 512]) for m in range(2)]
        ps_l2 = [ps(f"psl2_{m}", [128, 512]) for m in range(2)]
        ps_gate = [ps(f"psg{m}", [128, 512]) for m in range(4)]
        acc = [ps_l1[0], ps_l1[1]]
        ps_tr = ps_l2[0]

        s_w0 = sem("s_w0")
        s_in = [sem("s_in0"), sem("s_in1")]
        s_l1mm = sem("s_l1mm")
        s_l1ev = sem("s_l1ev")
        s_l2mm = sem("s_l2mm")
        s_l2ev = sem("s_l2ev")
        s_gmm = sem("s_gmm")
        s_gev = sem("s_gev")
        s_gw = sem("s_gw")
        s_gld = [sem("s_gld0"), sem("s_gld1")]
        s_pf = sem("s_pf")
        s_mm = sem("s_mm")
        s_sig = sem("s_sig")
        s_t2u = sem("s_t2u")
        s_tr = sem("s_tr")
        s_cT = sem("s_cT")
        s_d = sem("s_d")
        s_m1 = sem("s_m1")
        s_hT = sem("s_hT")
        s_prep = sem("s_prep")
        rsem = [sem("rsem0"), sem("rsem1")]
        lsem = [sem("lsem0"), sem("lsem1")]
        s_hst = [sem("s_hst0"), sem("s_hst1")]
        s_p3in = [sem("s_p3in0"), sem("s_p3in1")]
        s_p3mm = sem("s_p3mm")
        s_p3ev = sem("s_p3ev")
        s_p3st = sem("s_p3st")

        NW0 = 8 + 6 + 1 + 2 + 12 + 1 + 1 + 1 + 1

        def rthr(t):
            return 16 * (t // 2 + 1)

        with nc.Block() as block:

            @block.sync
            def _(sync):
                for k in range(8):
                    sync.dma_start(out=sb_whh[k][:, :], in_=whhT[128 * k:128 * (k + 1), :]).then_inc(s_w0, 16)
                for k in range(6):
                    sync.dma_start(out=sb_wih[k][:, :], in_=wihT[128 * k:128 * (k + 1), :]).then_inc(s_w0, 16)
                sync.dma_start(out=sb_pw1[:, :], in_=pw1T[:, :]).then_inc(s_w0, 16)
                for k in range(2):
                    sync.dma_start(out=sb_pw2[k][:, :], in_=pw2T[128 * k:128 * (k + 1), :]).then_inc(s_w0, 16)
                for k in range(12):
                    sync.dma_start(out=sb_proj[:, 80 * k:80 * (k + 1)], in_=projT[128 * k:128 * (k + 1), :]).then_inc(s_w0, 16)
                sync.dma_start(out=sb_biasg[:, :], in_=biasg[:, :]).then_inc(s_w0, 16)
                sync.dma_start(out=sb_projb[:, :], in_=projb[:, :]).then_inc(s_w0, 16)
                sync.dma_start(out=sb_ident[:, :], in_=ident[:, :]).then_inc(s_w0, 16)
                sync.dma_start(out=sb_cid[:, :], in_=cid[:, :]).then_inc(s_w0, 16)

                for j in range(NBLK):
                    jj = j % 2
                    c0 = 512 * j
                    if j >= 2:
                        sync.wait_ge(s_gmm, 24 * (j - 1))
                    sync.dma_start(out=sb_prev[jj][:, :], in_=prevT[:, c0:c0 + 512]).then_inc(s_in[jj], 16)
                    for k in range(4):
                        sync.dma_start(out=sb_mem[jj][k][:, :], in_=memT[128 * k:128 * (k + 1), c0:c0 + 512]).then_inc(s_in[jj], 16)
                    if j >= 1:
                        pc0 = 512 * (j - 1)
                        for m in range(4):
                            sync.wait_ge(s_gev, 4 * (j - 1) + m + 1)
                            sync.dma_start(out=g_dram[m, :, pc0:pc0 + 512], in_=sb_gout[(j - 1) % 2][m][:, :]).then_inc(s_gw, 16)
                pc0 = 512 * (NBLK - 1)
                for m in range(4):
                    sync.wait_ge(s_gev, 4 * (NBLK - 1) + m + 1)
                    sync.dma_start(out=g_dram[m, :, pc0:pc0 + 512], in_=sb_gout[(NBLK - 1) % 2][m][:, :]).then_inc(s_gw, 16)

                sync.wait_ge(s_gw, 64 * NBLK)
                for j in range(min(2, NBLK)):
                    for m in range(4):
                        sync.dma_start(out=sb_g[j % 2][m][:, :], in_=g_dram[m, :, 512 * j:512 * (j + 1)]).then_inc(s_gld[j % 2], 16)
                for t in range(T):
                    if t >= 16 and t % 16 == 0 and (t // 16 + 1) < NBLK:
                        j = t // 16 + 1
                        sync.wait_ge(s_pf, t)
                        for m in range(4):
                            sync.dma_start(out=sb_g[j % 2][m][:, :], in_=g_dram[m, :, 512 * j:512 * (j + 1)]).then_inc(s_gld[j % 2], 16)
                    sync.wait_ge(rsem[t % 2], rthr(t))
                    sync.dma_start(out=hs_dram[:, t, :], in_=sb_hall[t % 2][:, :]).then_inc(s_hst[t % 2], 16)

                my_id = nc.values_load(sb_cid[0:1, 0:1], engines=[mybir.EngineType.SP],
                                       min_val=0, max_val=7, skip_runtime_bounds_check=True)
                row0 = my_id * TC
                col0 = my_id * (TC * B)
                for n in range(P3N):
                    nt = min(16, TC - 16 * n)
                    ncols = nt * B
                    jj = n % 2
                    if n >= 2:
                        sync.wait_ge(s_p3mm, n - 1)
                    sync.dma_start(out=sb_hs[jj][:, 0:nt * 256],
                                   in_=hs_dram[:, DS(row0 + 16 * n, nt), :]).then_inc(s_p3in[jj], 16)
                    for k in range(4):
                        sync.dma_start(out=sb_m3[jj][k][:, 0:ncols],
                                       in_=memT[128 * k:128 * (k + 1), DS(col0 + 512 * n, ncols)]).then_inc(s_p3in[jj], 16)
                    sync.wait_ge(s_p3ev, n + 1)
                    sync.dma_start(out=melT[:, 512 * n:512 * n + ncols], in_=sb_mel[jj][:, 0:ncols]).then_inc(s_p3st, 16)
                sync.wait_ge(s_p3st, 16 * P3N)

            @block.gpsimd
            def _(gp):
                gp.wait_ge(s_w0, 16 * NW0)
                my_id = nc.values_load(sb_cid[0:1, 0:1], engines=[mybir.EngineType.Pool],
                                       min_val=0, max_val=7, skip_runtime_bounds_check=True)
                off = my_id * B
                for t in range(T):
                    gp.remote_dma_broadcast(
                        out_ap=sb_hall[t % 2][:, DS(off, B)],
                        in_ap=sb_hT[t % 2][:, :],
                        remote_sem=rsem[t % 2],
                        local_sem=lsem[t % 2],
                        rdests=[(0, k) for k in range(8)],
                    ).then_inc(s_prep, 1)
                    gp.wait_ge(s_prep, t + 1)
                    gp.wait_ge(s_hT, t + 1)
                    if t >= 1:
                        gp.wait_ge(rsem[(t - 1) % 2], rthr(t - 1))
                        gp.wait_ge(s_hst[(t - 1) % 2], 16 * ((t - 1) // 2 + 1))
                    gp.trigger_dma(count=1)
                gp.wait_ge(rsem[(T - 1) % 2], rthr(T - 1))

            @block.tensor
            def _(pe):
                pe.wait_ge(s_w0, 16 * NW0)
                for j in range(NBLK):
                    jj = j % 2
                    pe.wait_ge(s_in[jj], 80 * (j // 2 + 1))
                    if j >= 1:
                        pe.wait_ge(s_l1ev, 2 * j)
                    for m in range(2):
                        pe.matmul(ps_l1[m][:, :], r32(sb_pw1[:, 128 * m:128 * (m + 1)]),
                                  r32(sb_prev[jj][:, :]), start=True, stop=True).then_inc(s_l1mm, 1)
                    pe.wait_ge(s_l1ev, 2 * (j + 1))
                    if j >= 1:
                        pe.wait_ge(s_l2ev, 2 * j)
                    for m in range(2):
                        for k in range(2):
                            i = pe.matmul(ps_l2[m][:, :], r32(sb_pw2[k][:, 128 * m:128 * (m + 1)]),
                                          r32(sb_h1[jj][k][:, :]), start=(k == 0), stop=(k == 1))
                        i.then_inc(s_l2mm, 1)
                    pe.wait_ge(s_l2ev, 2 * (j + 1))
                    if j >= 1:
                        pe.wait_ge(s_gev, 4 * j)
                    for m in range(4):
                        for k in range(2):
                            pe.matmul(ps_gate[m][:, :], r32(sb_wih[k][:, 128 * m:128 * (m + 1)]),
                                      r32(sb_h2[jj][k][:, :]), start=(k == 0), stop=False).then_inc(s_gmm, 1)
                        for k in range(4):
                            pe.matmul(ps_gate[m][:, :], r32(sb_wih[2 + k][:, 128 * m:128 * (m + 1)]),
                                      r32(sb_mem[jj][k][:, :]), start=False, stop=(k == 3)).then_inc(s_gmm, 1)

                for t in range(T):
                    a = acc[t % 2]
                    jj = (t // 16) % 2
                    col = (t % 16) * B
                    jb = t // 16
                    pe.wait_ge(s_gld[jb % 2], 64 * (jb // 2 + 1))
                    if t >= 2:
                        pe.wait_ge(s_sig, 2 * (t - 1))
                    for m in range(4):
                        i = pe.matmul(r32(a[0:B, 128 * m:128 * (m + 1)]),
                                      r32(sb_g[jj][m][:, col:col + B]), r32(sb_ident[:, :]),
                                      start=(m == 0), stop=(t == 0 and m == 3),
                                      is_transpose=True, skip_group_check=True)
                    i.then_inc(s_pf, 1)
                    if t >= 1:
                        pe.wait_ge(rsem[(t - 1) % 2], rthr(t - 1))
                        for k in range(8):
                            i = pe.matmul(a[0:B, :], r32(sb_hall[(t - 1) % 2][:, B * k:B * (k + 1)]),
                                          r32(sb_whh[k][:, :]), start=False, stop=(k == 7),
                                          skip_group_check=True)
                        i.then_inc(s_mm, 1)
                    pe.wait_ge(s_sig, 2 * t + 1)
                    pe.matmul(r32(ps_tr[:, 0:32]), r32(sb_sig[:, 128:256]), r32(sb_ident[0:B, 0:B]),
                              start=True, stop=True, is_transpose=True, skip_group_check=True)
                    pe.matmul(r32(ps_tr[:, 32:64]), r32(sb_sig[:, 256:384]), r32(sb_ident[0:B, 0:B]),
                              start=True, stop=True, is_transpose=True, skip_group_check=True)
                    pe.wait_ge(s_t2u, t + 1)
                    pe.matmul(r32(ps_tr[:, 64:96]), r32(sb_t2u[:, :]), r32(sb_ident[0:B, 0:B]),
                              start=True, stop=True, is_transpose=True,
                              skip_group_check=True).then_inc(s_tr, 1)

                for n in range(P3N):
                    nt = min(16, TC - 16 * n)
                    ncols = nt * B
                    jj = n % 2
                    pe.wait_ge(s_p3in[n % 2], 80 * (n // 2 + 1))
                    if n >= 1:
                        pe.wait_ge(s_p3ev, n)
                    hsv = sb_hs[jj].ap().rearrange("p (t c) -> p t c", c=256)
                    for k in range(8):
                        pe.matmul(ps_gate[0][0:N_MEL, 0:ncols],
                                  r32(sb_proj[:, 80 * k:80 * (k + 1)]),
                                  r32(hsv[:, 0:nt, B * k:B * (k + 1)]),
                                  start=(k == 0), stop=False, skip_group_check=True)
                    for k in range(4):
                        i = pe.matmul(ps_gate[0][0:N_MEL, 0:ncols],
                                      r32(sb_proj[:, 80 * (8 + k):80 * (9 + k)]),
                                      r32(sb_m3[jj][k][:, 0:ncols]),
                                      start=False, stop=(k == 3), skip_group_check=True)
                    i.then_inc(s_p3mm, 1)

            @block.scalar
            def _(act):
                for j in range(NBLK):
                    jj = j % 2
                    for m in range(4):
                        act.wait_ge(s_gmm, 24 * j + 6 * (m + 1))
                        act.activation(sb_gout[jj][m][:, :], ps_gate[m][:, :], AF.Identity,
                                       bias=sb_biasg[:, m:m + 1]).then_inc(s_gev, 1)
                for t in range(T):
                    a = acc[t % 2]
                    if t == 0:
                        act.wait_ge(s_pf, 1)
                    else:
                        act.wait_ge(s_mm, t)
                        act.wait_ge(s_tr, t)
                        act.wait_ge(s_t2u, t)
                    act.activation(sb_sig[:, :], a[0:B, 0:384], AF.Sigmoid).then_inc(s_sig, 1)
                    act.activation(sb_tg[:, :], a[0:B, 384:512], AF.Tanh).then_inc(s_sig, 1)
                    act.wait_ge(s_cT, t + 1)
                    if t >= 1:
                        act.wait_ge(s_hT, t)
                    act.activation(sb_d[:, :], sb_c[t % 2][:, :], AF.Tanh).then_inc(s_d, 1)
                for n in range(P3N):
                    nt = min(16, TC - 16 * n)
                    ncols = nt * B
                    act.wait_ge(s_p3mm, n + 1)
                    act.activation(sb_mel[n % 2][:, 0:ncols], ps_gate[0][0:N_MEL, 0:ncols],
                                   AF.Identity, bias=sb_projb[:, :]).then_inc(s_p3ev, 1)

            @block.vector
            def _(vec):
                for j in range(NBLK):
                    jj = j % 2
                    for m in range(2):
                        vec.wait_ge(s_l1mm, 2 * j + m + 1)
                        vec.tensor_relu(sb_h1[jj][m][:, :], ps_l1[m][:, :]).then_inc(s_l1ev, 1)
                    for m in range(2):
                        vec.wait_ge(s_l2mm, 2 * j + m + 1)
                        vec.tensor_relu(sb_h2[jj][m][:, :], ps_l2[m][:, :]).then_inc(s_l2ev, 1)
                for t in range(T):
                    vec.wait_ge(s_sig, 2 * t + 2)
                    if t >= 1:
                        vec.wait_ge(s_tr, t)
                    vec.tensor_mul(sb_t2u[:, :], sb_sig[:, 0:128], sb_tg[:, :]).then_inc(s_t2u, 1)
                    vec.wait_ge(s_tr, t + 1)
                    if t == 0:
                        vec.tensor_copy(sb_c[0][:, :], ps_tr[:, 64:96]).then_inc(s_cT, 1)
                    else:
                        vec.wait_ge(s_cT, t)
                        vec.tensor_mul(sb_m1[:, :], ps_tr[:, 0:32], sb_c[(t - 1) % 2][:, :]).then_inc(s_m1, 1)
                        vec.wait_ge(s_m1, t)
                        vec.tensor_add(sb_c[t % 2][:, :], sb_m1[:, :], ps_tr[:, 64:96]).then_inc(s_cT, 1)
                    vec.wait_ge(s_d, t + 1)
                    if t >= 2:
                        vec.wait_ge(lsem[t % 2], 16 * (t // 2))
                    vec.tensor_mul(sb_hT[t % 2][:, :], ps_tr[:, 32:64], sb_d[:, :]).then_inc(s_hT, 1)

    nc.compile()
    return nc


def _stage_inputs(inputs, T):
    memory = np.asarray(inputs["memory"], np.float32)[:, :T]
    y_mels = np.asarray(inputs["y_mels"], np.float32)[:, :T]
    w1 = np.asarray(inputs["prenet_w1"], np.float32)
    w2 = np.asarray(inputs["prenet_w2"], np.float32)
    w_ih = np.asarray(inputs["w_ih"], np.float32)
    w_hh = np.asarray(inputs["w_hh"], np.float32)
    bsum = np.asarray(inputs["b_ih"], np.float32) + np.asarray(inputs["b_hh"], np.float32)
    proj_w = np.asarray(inputs["proj_w"], np.float32)
    proj_b = np.asarray(inputs["proj_b"], np.float32)

    prev = np.concatenate([np.zeros((B, 1, N_MEL), np.float32), y_mels[:, : T - 1]], axis=1)
    prevT = np.ascontiguousarray(prev.transpose(2, 1, 0)).reshape(N_MEL, T * B)
    memT = np.ascontiguousarray(memory.transpose(2, 1, 0)).reshape(ATTN, T * B)
    pw1T = np.ascontiguousarray(w1.T)
    pw2T = np.ascontiguousarray(w2.T)
    projT = np.ascontiguousarray(proj_w.T)
    projb = np.ascontiguousarray(proj_b[:, None])
    ident = np.eye(128, dtype=np.float32)

    cores = []
    for c in range(NCORE):
        rows = np.concatenate([
            np.arange(128 * c, 128 * (c + 1)),
            1024 + np.arange(128 * c, 128 * (c + 1)),
            3072 + np.arange(128 * c, 128 * (c + 1)),
            2048 + np.arange(128 * c, 128 * (c + 1)),
        ])
        cores.append(dict(
            prevT=prevT, memT=memT, pw1T=pw1T, pw2T=pw2T,
            wihT=np.ascontiguousarray(w_ih[rows].T),
            whhT=np.ascontiguousarray(w_hh[rows].T),
            biasg=np.ascontiguousarray(bsum[rows].reshape(4, 128).T),
            projT=projT, projb=projb, ident=ident,
            core_id=np.array([[c]], np.uint32),
        ))
    return cores


def kernel(**inputs) -> np.ndarray:
    from concourse.bass_utils import run_bass_kernel_spmd

    T = int(np.asarray(inputs["memory"]).shape[1])
    if T not in _nc_cache:
        _nc_cache[T] = _build(T)
    nc = _nc_cache[T]
    core_ins = _stage_inputs(inputs, T)
    res = run_bass_kernel_spmd(nc, core_ins, core_ids=list(range(NCORE)))
    TC = T // NCORE
    chunks = []
    for c in range(NCORE):
        m = np.asarray(res.results[c]["melT"]).reshape(N_MEL, TC, B)
        chunks.append(m.transpose(2, 1, 0))
    return np.ascontiguousarray(np.concatenate(chunks, axis=1))
